# revision 44
# baseline (speedup 1.0000x reference)
"""Trainium2 Bass kernel for the 5x5 nearest-neighbor L1 loss.

Reference semantics (B=16, C=3, H=W=256):
    pad gt spatially by 2 with BIG, take the 25 shifted views,
    per pixel: min over the 25 shifts of sum_c |gt_shift - pred|,
    return the scalar sum over all pixels.

Sharding: pure data parallel over batch: 2 images per NeuronCore x 8 cores,
each core returns a partial scalar sum; the host adds the 8 partials.

Per-core layout (all fp16, converted/marshalled on host):
    partitions p = (b, g): b in [0,2), g in [0,64)   -> 128 partitions
    pred   P[p, j, c, w]   = pred[b, c, 4g + j, w]          j in [0,4)
    g_even G[p, jj, c, w'] = gt_pad[b, c, 4g + jj, w']      jj in [0,8)
    g_odd  same but columns shifted one to the left (source col w'+1)
where gt_pad is gt padded by 2 rows/cols of BIG (out-of-image -> BIG).

Key layout properties:
  * Halo in the free dim: the 8 jj-rows per partition cover padded rows
    4g..4g+7, so for output row j and vertical shift o in [-2,2] the
    source row jj = j + o + 2 stays inside the partition -- every one of
    the 25 shifts is a pure free-dim offset, no partition crossing.
  * (jj, c) adjacent with c inner: a shift's gather of rows
    (jj0..jj0+3) x (all c) is 12 consecutive (jj,c) pairs = ONE
    uniform-stride AP dim, so every op uses <=2 free dims (small ISA
    encodings -- TRN2 engine instructions fit only one sync wait).
  * The odd-column copy keeps the W-shift column offsets even, i.e.
    4-byte aligned, which the DVE's packed-fp16 perf modes require.

Per shift (25 total) the device does:
    d = G_view - P               (DVE tensor_tensor fp16)
    d = |d|                      (ScalarE Abs in place, parallel to DVE)
    s = d_c0 + d_c1; s2 = s+d_c2 (DVE adds = channel sum)
    m = min(m, s2)               (DVE min accumulate)
then reduces m over the free dim (fp32), sums partitions with a 128x1
matmul against ones, and DMAs the [1,1] fp32 partial out.

Measured on TRN2 (paired differential wall-clock over repeat variants):
the 25-shift body runs at ~40 us/core, which is the DVE 2-read-port
packed-fp16 streaming floor for the 6144 two-input elementwise ops per
shift-pixel-lane; ScalarE |.| runs just under that in parallel.
"""

import numpy as np

NCORES = 8
BIG = np.float16(20000.0)

GE_COLS = 3 * 8 * 260
P_COLS = 3 * 4 * 256

_cache = {}


def _build_nc(repeat=1, dma_min=False, bufs=3, n_dve_abs=0):
    from contextlib import ExitStack

    import concourse.bacc as bacc
    import concourse.mybir as mybir
    import concourse.tile as tile

    f16 = mybir.dt.float16
    f32 = mybir.dt.float32
    Alu = mybir.AluOpType

    # Bacc (not raw Bass): its finalize() runs generate_event_semaphores,
    # which splits multi-sem waits into event-semaphore instructions --
    # TRN2 engine instructions encode at most ONE sync wait (NCC_INLA001
    # "Too many sync wait commands" otherwise).
    nc = bacc.Bacc("TRN2", target_bir_lowering=False)
    ge_d = nc.dram_tensor("g_even", [128, GE_COLS], f16, kind="ExternalInput")
    go_d = nc.dram_tensor("g_odd", [128, GE_COLS], f16, kind="ExternalInput")
    p_d = nc.dram_tensor("pred", [128, P_COLS], f16, kind="ExternalInput")
    out_d = nc.dram_tensor("out", [1, 1], f32, kind="ExternalOutput")

    with ExitStack() as ctx:
        tc = ctx.enter_context(tile.TileContext(nc))
        pool = ctx.enter_context(tc.tile_pool(name="main", bufs=1))
        dpool = ctx.enter_context(tc.tile_pool(name="work", bufs=bufs))
        pspool = ctx.enter_context(tc.tile_pool(name="ps", bufs=1, space="PSUM"))

        ge = pool.tile([128, GE_COLS], f16, tag="ge")
        go = pool.tile([128, GE_COLS], f16, tag="go")
        p = pool.tile([128, P_COLS], f16, tag="p")
        m = pool.tile([128, 1024], f16, tag="m")
        n_acc = dma_min if isinstance(dma_min, int) and dma_min > 1 else 1
        if n_acc > 1:
            m_acc = [m]
            for i in range(1, n_acc):
                mx = pool.tile([128, 1024], f16, tag=f"m{i}")
                m_acc.append(mx)
            acc_init = [False] * n_acc

        # Load order matches consumption order (even-dj shifts run first and
        # scan windows bottom-up): low half of g_even, pred, high half of
        # g_even, then g_odd streaming in behind the even-dj compute. Tile's
        # subtile dep tracking lets the early subs start before the rest of
        # the tile lands.
        half = 12 * 260
        nc.sync.dma_start(out=ge[:, 0:half], in_=ge_d[:, 0:half])
        nc.sync.dma_start(out=p[:], in_=p_d[:])
        nc.sync.dma_start(out=ge[:, half : 2 * half], in_=ge_d[:, half : 2 * half])
        nc.sync.dma_start(out=go[:], in_=go_d[:])

        # pred viewed as [p, (j c), w]: 12 rows of width 256
        pv3 = p[:].rearrange("p (r w) -> p r w", r=12, w=256)

        # g tiles viewed as [p, (jj c), w']: 24 rows of width 260
        gev = ge[:].rearrange("p (r w) -> p r w", r=24, w=260)
        gov = go[:].rearrange("p (r w) -> p r w", r=24, w=260)

        # Even-dj shifts first: they only read g_even, so the g_odd DMA
        # overlaps their compute instead of blocking the pipeline.
        one_pass = [(o, dj) for o in range(-2, 3) for dj in (0, 2, 4)] + [
            (o, dj) for o in range(-2, 3) for dj in (1, 3)
        ]
        shifts = one_pass * repeat
        for idx, (o, dj) in enumerate(shifts):
            par = dj % 2
            src = gov if par else gev
            col = dj - par
            r0 = (o + 2) * 3  # first (jj, c) row of the 12-row window
            gsl = src[:, r0 : r0 + 12, col : col + 256]

            d = dpool.tile([128, 3072], f16, tag="d")
            dv = d[:].rearrange("p (r w) -> p r w", r=12, w=256)
            nc.vector.tensor_tensor(dv, gsl, pv3, Alu.subtract)

            # |d| in place on the scalar engine (runs parallel to DVE).
            a = d
            if (idx % 25) < n_dve_abs:
                nc.vector.tensor_scalar(a[:], d[:], 0.0, None, Alu.abs_max)
            else:
                nc.scalar.activation(a[:], d[:], mybir.ActivationFunctionType.Abs)

            # channel views of a: [p, (j:4, stride 768), (w:256)] at offset c*256
            av = a[:].rearrange("p (j c w) -> p j c w", j=4, c=3, w=256)
            a_c0 = av[:, :, 0, :]
            a_c1 = av[:, :, 1, :]
            a_c2 = av[:, :, 2, :]

            s = dpool.tile([128, 1024], f16, tag="s")
            sv = s[:].rearrange("p (j w) -> p j w", j=4, w=256)
            nc.vector.tensor_tensor(sv, a_c0, a_c1, Alu.add)
            if n_acc > 1:
                k = idx % n_acc
                tgt = m_acc[k]
                tv = tgt[:].rearrange("p (j w) -> p j w", j=4, w=256)
                if not acc_init[k]:
                    nc.vector.tensor_tensor(tv, sv, a_c2, Alu.add)
                    acc_init[k] = True
                else:
                    s2 = dpool.tile([128, 1024], f16, tag="s2")
                    s2v = s2[:].rearrange("p (j w) -> p j w", j=4, w=256)
                    nc.vector.tensor_tensor(s2v, sv, a_c2, Alu.add)
                    nc.gpsimd.dma_start(out=tgt[:], in_=s2[:], accum_op=Alu.min)
                continue
            mv = m[:].rearrange("p (j w) -> p j w", j=4, w=256)
            if idx == 0:
                nc.vector.tensor_tensor(mv, sv, a_c2, Alu.add)
            else:
                s2 = dpool.tile([128, 1024], f16, tag="s2")
                s2v = s2[:].rearrange("p (j w) -> p j w", j=4, w=256)
                nc.vector.tensor_tensor(s2v, sv, a_c2, Alu.add)
                if dma_min:
                    nc.gpsimd.dma_start(out=m[:], in_=s2[:], accum_op=Alu.min)
                else:
                    nc.vector.tensor_tensor(m[:], m[:], s2[:], Alu.min)

        if n_acc > 1:
            for i in range(1, n_acc):
                nc.vector.tensor_tensor(m[:], m[:], m_acc[i][:], Alu.min)
        r1 = pool.tile([128, 1], f32, tag="r1")
        nc.vector.tensor_reduce(r1[:], m[:], mybir.AxisListType.X, Alu.add)
        ones = pool.tile([128, 1], f32, tag="ones")
        nc.vector.memset(ones[:], 1.0)
        ps = pspool.tile([1, 1], f32)
        nc.tensor.matmul(ps[:], r1[:], ones[:], start=True, stop=True)
        osb = pool.tile([1, 1], f32, tag="osb")
        nc.scalar.copy(osb[:], ps[:])
        nc.sync.dma_start(out=out_d[:], in_=osb[:])

    if not nc.is_finalized():
        nc.finalize()
    return nc


def marshal_core(pred2, gt2):
    """pred2, gt2: [2, 3, 256, 256] f32 -> core input dict (fp16 layouts)."""
    gtp = np.full((2, 3, 260, 262), BIG, np.float16)
    gtp[:, :, 2:258, 2:258] = gt2.astype(np.float16)
    sw = np.lib.stride_tricks.sliding_window_view(gtp, 8, axis=2)  # [2,3,253,262,8]
    sel = sw[:, :, 0:253:4]  # rows 4g -> [2,3,64,262,8] = (b,c,g,w,jj)
    base = sel.transpose(0, 2, 4, 1, 3)  # (b,g,jj,c,w) = [2,64,8,3,262]
    ge = np.ascontiguousarray(base[..., 0:260]).reshape(128, GE_COLS)
    go = np.ascontiguousarray(base[..., 1:261]).reshape(128, GE_COLS)
    p16 = (
        pred2.astype(np.float16)
        .reshape(2, 3, 64, 4, 256)  # (b,c,g,j,w)
        .transpose(0, 2, 3, 1, 4)  # (b,g,j,c,w)
        .reshape(128, P_COLS)
    )
    return {
        "g_even": ge,
        "g_odd": go,
        "pred": np.ascontiguousarray(p16),
    }


_last_results = None


def kernel(pred_target, gt_target):
    global _last_results
    from concourse.bass_utils import run_bass_kernel_spmd

    pred_target = np.asarray(pred_target)
    gt_target = np.asarray(gt_target)

    if "nc" not in _cache:
        _cache["nc"] = _build_nc()
    nc = _cache["nc"]

    in_maps = [
        marshal_core(pred_target[2 * ci : 2 * ci + 2], gt_target[2 * ci : 2 * ci + 2])
        for ci in range(NCORES)
    ]
    try:
        res = run_bass_kernel_spmd(nc, in_maps, core_ids=list(range(NCORES)))
    except ModuleNotFoundError:
        # BASS_TRACE was requested but this container has no axon NTFF
        # hook (antenv.axon_hooks) -- run without tracing.
        import os

        os.environ["BASS_NEVER_TRACE"] = "1"
        res = run_bass_kernel_spmd(nc, in_maps, core_ids=list(range(NCORES)))
    _last_results = res
    total = 0.0
    for r in res.results:
        total += float(r["out"][0, 0])
    return np.float32(total)
